# revision 1
# baseline (speedup 1.0000x reference)
"""ChiSquareLoss kernel for Trainium2 (8 NeuronCores, SPMD).

Problem (see reference): for each of B=16384 rows of a [B, 2048] f32 matrix,
build a 10-bin histogram between the row's min and max, then
chi2_row = sum_j (obs_j - e)^2 / (e + eps) with e = B/10, and return
mean(chi2_row).

Algorithm per row (each core handles B/8 = 2048 rows):
  searchsorted(boundaries, x, side='left') == #boundaries strictly below x,
  so the bin index is #{k in 1..9 : x > b_k}, b_k = mn + (mx-mn)*k/10.
  The histogram comes from cumulative counts c_k = #{x in row : x > b_k}:
  obs_j = c_j - c_{j+1}, c_0 = 2048, c_10 = 0.

Accumulated (reduction) ops run at 1x on DVE/ACT (~2.2us / [128,2048] tile),
so DVE packs TWO counts per accumulated pass:
  pair(lo,hi):  mask = (xb > b_hi)*4096   (bf16 tensor_scalar, 4x mode,
                 xb = bf16 copy of x from a casting DMA)
                acc  = sum((x > b_lo) + mask)  (scalar_tensor_tensor, 1x)
  -> acc = c_lo + 4096*c_hi, exact in fp32 (c <= 2047, sum < 2^24).
c_lo is exact fp32; c_hi classifies bf16(x) vs b_hi, which only perturbs
elements within half a bf16 ulp of a boundary (~0.4/row, random sign;
measured end-to-end relative error ~2e-6).

Engine split per [128, 2048] tile:
  DVE : row max, row min (tensor_scalar + max/min accumulator),
        pair-counts (1,2),(3,4); last two tiles also (5,6)
  ACT : boundary vectors (delta, b_k via Identity with AP scale/bias),
        remaining counts via Sign(b_k - x) + sum accumulator
        (scale=-1 avoids materializing -b_k)
The last two tiles shift two counts from ACT to DVE because ACT runs one
min/max pass behind DVE; this drains the phase lag at the end.
Epilogue: unpack pairs (floor via the 2^23 magic constant), convert ACT
sign-sums to counts, difference into obs, one ACT Square(obs - e) pass with
accumulator -> per-partition partial sums.  Host: total / (e + eps) / B.
"""

import numpy as np

_B_FULL = 16384
_D = 2048
_N_CORES = 8
_ROWS_PER_CORE = _B_FULL // _N_CORES  # 2048
_P = 128
_TILES = _ROWS_PER_CORE // _P  # 16
_BINS = 10
# reference: expected = f32(B/BINS); expected + 1e-8 rounds back to the same f32
_E_F32 = np.float32(_B_FULL / _BINS)  # 1638.4f

_MAGIC = float(np.float32(2 ** 23 + 2 ** 22))  # round-to-int magic for fp32
_N_TAIL = 2  # trailing tiles that take 3 pairs on DVE

_CACHE = {}


def _build_program():
    import concourse.bacc as bacc
    import concourse.mybir as mybir
    import concourse.tile as tile

    f32 = mybir.dt.float32
    bf16 = mybir.dt.bfloat16
    Alu = mybir.AluOpType
    Act = mybir.ActivationFunctionType

    nc = bacc.Bacc(None, target_bir_lowering=False)
    x = nc.dram_tensor("x", [_ROWS_PER_CORE, _D], f32, kind="ExternalInput")
    out = nc.dram_tensor("partial", [_P, 1], f32, kind="ExternalOutput")

    T = _TILES
    TB = T - _N_TAIL
    # fracs exactly as the reference: f32(k)/f32(10)
    fr = [float(np.float32(k) / np.float32(10.0)) for k in range(1, 10)]

    with tile.TileContext(nc) as tc:
        with tc.tile_pool(name="singles", bufs=1) as singles, \
             tc.tile_pool(name="xp", bufs=4) as xpool, \
             tc.tile_pool(name="dscr", bufs=3) as dscr, \
             tc.tile_pool(name="mscr", bufs=3) as mscr, \
             tc.tile_pool(name="pscr", bufs=3) as pscr, \
             tc.tile_pool(name="ascr", bufs=3) as ascr, \
             tc.tile_pool(name="small", bufs=4) as small:

            # persistent accumulators
            pairacc = singles.tile([_P, T * 3], f32)   # 3 pair slots / tile
            sgnacc = singles.tile([_P, T * 5], f32)    # 5 sign slots / tile
            c_all = singles.tile([_P, T * 11], f32)    # c_0..c_10 per tile
            fracs = singles.tile([_P, 9], f32)         # k/10
            ebias = singles.tile([_P, 1], f32)         # -e
            c3 = c_all[:].rearrange("p (t k) -> p t k", k=11)
            nc.gpsimd.memset(c3[:, :, 0:1], float(_D))   # c_0 = 2048
            nc.gpsimd.memset(c3[:, :, 10:11], 0.0)       # c_10 = 0
            for i, f in enumerate(fr):
                nc.gpsimd.memset(fracs[:, i:i + 1], f)
            nc.gpsimd.memset(ebias[:], -float(_E_F32))
            pa3 = pairacc[:].rearrange("p (t k) -> p t k", k=3)
            nc.gpsimd.memset(pa3[:, 0:TB, 2:3], 0.0)     # unused 3rd pair slot
            sg3 = sgnacc[:].rearrange("p (t k) -> p t k", k=5)
            nc.gpsimd.memset(sg3[:, TB:T, 3:5], 0.0)     # tail unused sign slots

            def counts_for(t, st):
                xt, xb, bpos = st
                tail = t >= TB
                pairs_t = [(1, 2), (3, 4), (5, 6)] if tail else [(1, 2), (3, 4)]
                act_ks = [7, 8, 9] if tail else [5, 6, 7, 8, 9]
                for pi, (lo, hi) in enumerate(pairs_t):
                    mhi = mscr.tile([_P, _D], bf16, tag="mask")
                    nc.vector.tensor_scalar(mhi[:], xb[:], bpos[:, hi - 1:hi],
                                            4096.0, Alu.is_gt, Alu.mult)
                    sp = pscr.tile([_P, _D], f32, tag="pair")
                    col = t * 3 + pi
                    nc.vector.scalar_tensor_tensor(
                        out=sp[:], in0=xt[:], scalar=bpos[:, lo - 1:lo],
                        in1=mhi[:], op0=Alu.is_gt, op1=Alu.add,
                        accum_out=pairacc[:, col:col + 1])
                for i, k in enumerate(act_ks):
                    slot = t * 5 + i
                    s = ascr.tile([_P, _D], bf16, tag="actscr")
                    nc.scalar.activation(
                        s[:], xt[:], Act.Sign, bias=bpos[:, k - 1:k], scale=-1.0,
                        accum_out=sgnacc[:, slot:slot + 1])

            # one-tile software pipeline: emit tile t's counts after tile
            # t+1's min/max + boundary ops so neither engine waits on the
            # cross-engine boundary chain (DVE minmax -> ACT b_k -> DVE pairs)
            prev = None
            for t in range(T):
                xt = xpool.tile([_P, _D], f32, tag="xt")
                nc.sync.dma_start(out=xt[:], in_=x[t * _P:(t + 1) * _P, :])

                mx = small.tile([_P, 1], f32, tag="mx")
                mn = small.tile([_P, 1], f32, tag="mn")
                delta = small.tile([_P, 1], f32, tag="delta")
                bpos = small.tile([_P, 9], f32, tag="bpos")  # b_k

                # row max; the pass-through output doubles as bf16(x)
                xb = dscr.tile([_P, _D], bf16, tag="xbscr")
                nc.vector.tensor_scalar(xb[:], xt[:], 1.0, None,
                                        Alu.mult, Alu.max, accum_out=mx[:])
                s_mm2 = dscr.tile([_P, _D], bf16, tag="dvescr")
                nc.vector.tensor_scalar(s_mm2[:], xt[:], 1.0, None,
                                        Alu.mult, Alu.min, accum_out=mn[:])
                # boundary math on ACT: delta = -mn + mx ; b_k = frac_k*delta + mn
                nc.scalar.activation(delta[:], mn[:], Act.Identity,
                                     bias=mx[:], scale=-1.0)
                nc.scalar.activation(bpos[:], fracs[:], Act.Identity,
                                     bias=mn[:], scale=delta[:])

                if prev is not None:
                    counts_for(t - 1, prev)
                prev = (xt, xb, bpos)
            counts_for(T - 1, prev)

            # ---- epilogue ----
            # unpack pairs first (DVE-only deps; runs while ACT drains)
            chi = singles.tile([_P, T * 3], f32)
            clo = singles.tile([_P, T * 3], f32)
            nc.vector.tensor_scalar(chi[:], pairacc[:], float(2.0 ** -12),
                                    _MAGIC, Alu.mult, Alu.add)
            nc.vector.tensor_scalar(chi[:], chi[:], -_MAGIC, None, Alu.add)
            nc.vector.scalar_tensor_tensor(
                out=clo[:], in0=chi[:], scalar=-4096.0, in1=pairacc[:],
                op0=Alu.mult, op1=Alu.add)
            chi3 = chi[:].rearrange("p (t k) -> p t k", k=3)
            clo3 = clo[:].rearrange("p (t k) -> p t k", k=3)
            # pairs (1,2) and (3,4): every tile
            for pi, (lo, hi) in enumerate([(1, 2), (3, 4)]):
                nc.vector.tensor_copy(c3[:, :, lo:lo + 1], clo3[:, :, pi:pi + 1])
                nc.vector.tensor_copy(c3[:, :, hi:hi + 1], chi3[:, :, pi:pi + 1])
            # pair (5,6): tail tiles only
            nc.vector.tensor_copy(c3[:, TB:T, 5:6], clo3[:, TB:T, 2:3])
            nc.vector.tensor_copy(c3[:, TB:T, 6:7], chi3[:, TB:T, 2:3])
            # ACT sign-sums (sign(b_k - x)) -> counts: c = 1024 - 0.5*S
            conv = singles.tile([_P, T * 5], f32)
            nc.vector.tensor_scalar(conv[:], sgnacc[:], -0.5, float(_D // 2),
                                    Alu.mult, Alu.add)
            conv3 = conv[:].rearrange("p (t k) -> p t k", k=5)
            nc.vector.tensor_copy(c3[:, 0:TB, 5:10], conv3[:, 0:TB, 0:5])
            nc.vector.tensor_copy(c3[:, TB:T, 7:10], conv3[:, TB:T, 0:3])
            # obs_j = c_j - c_{j+1}
            obs = singles.tile([_P, T * 10], f32)
            obs3 = obs[:].rearrange("p (t j) -> p t j", j=10)
            nc.vector.tensor_tensor(out=obs3[:, :, 0:10], in0=c3[:, :, 0:10],
                                    in1=c3[:, :, 1:11], op=Alu.subtract)

            sq = singles.tile([_P, T * 10], f32)
            part = singles.tile([_P, 1], f32)
            nc.scalar.activation(sq[:], obs[:], Act.Square,
                                 bias=ebias[:], scale=1.0,
                                 accum_out=part[:])
            nc.sync.dma_start(out=out[:], in_=part[:])

    nc.compile()
    return nc


def _get_program():
    if "nc" not in _CACHE:
        _CACHE["nc"] = _build_program()
    return _CACHE["nc"]


def kernel(embeddings: np.ndarray) -> np.ndarray:
    from concourse.bass_utils import run_bass_kernel_spmd

    assert embeddings.shape == (_B_FULL, _D), embeddings.shape
    x = np.ascontiguousarray(embeddings, dtype=np.float32)
    nc = _get_program()
    in_maps = [
        {"x": x[c * _ROWS_PER_CORE:(c + 1) * _ROWS_PER_CORE]}
        for c in range(_N_CORES)
    ]
    res = run_bass_kernel_spmd(nc, in_maps, core_ids=list(range(_N_CORES)))
    total = np.float64(0.0)
    for r in res.results:
        total += r["partial"].astype(np.float64).sum()
    mean_chi2 = total / np.float64(_E_F32) / np.float64(_B_FULL)
    return np.float32(mean_chi2)

